# revision 21
# baseline (speedup 1.0000x reference)
"""Trainium2 Bass kernel for nn_BGNN_MLP (bipartite 3-layer GNN).

Self-contained: kernel(**inputs) -> np.ndarray takes the full unsharded
inputs and returns the full [50000, 128] output, running on 8 NeuronCores
via run_bass_kernel_spmd.

Algorithm (per layer l = 0,1,2; directions U,V,U):
  z = input @ W_l            (dense, per-core slice, node-major)
  publish z slice -> AllGather -> Z table [8*WV, 128] in DRAM
  aggregate: out[d] = sum_{edges e: dest(e)=d} z[src(e)]  + deg(d)*b_l
    via per-superbin gather tiles (dma_gather, 128 edge slots/tile) and
    PE matmuls (gathered rows stationary, 0/1 selector M moving) into
    PSUM windows; the bias enters as a rank-1 outer(b, deg) matmul that
    also initializes each window.

SPMD: one instruction stream for all 8 cores; all per-core variation is
carried by ExternalInput data (packing layout, gather indices, M, deg).
"""

import sys

if "/opt/trn_rl_repo" not in sys.path:
    sys.path.insert(0, "/opt/trn_rl_repo")

import numpy as np

NC = 8

# ----------------------------------------------------------------------------
# host-side packing
# ----------------------------------------------------------------------------


def _pack_core(lo_cnt, hi_cnt, wm):
    """2D FFD, imbalance-aware. Returns list of bins (lists of local ids)."""
    order = np.argsort(-(lo_cnt + hi_cnt), kind="stable")
    bins, bl, bh = [], [], []
    open_bins = []
    for li in order:
        li = int(li)
        l, h = int(lo_cnt[li]), int(hi_cnt[li])
        best, best_score = -1, None
        for bi in open_bins:
            if len(bins[bi]) >= wm:
                continue
            nl, nh = bl[bi] + l, bh[bi] + h
            if nl > 128 or nh > 128:
                continue
            score = abs(nl - nh)
            if best_score is None or score < best_score:
                best_score, best = score, bi
        if best < 0:
            bins.append([li]); bl.append(l); bh.append(h)
        else:
            bins[best].append(li); bl[best] += l; bh[best] += h
        bi = best if best >= 0 else len(bins) - 1
        if bi not in open_bins:
            if not (max(bl[bi], bh[bi]) > 127 or len(bins[bi]) >= wm):
                open_bins.append(bi)
        elif max(bl[bi], bh[bi]) > 127 or len(bins[bi]) >= wm:
            open_bins.remove(bi)
        if len(open_bins) > 48:
            fullest = max(open_bins, key=lambda b2: max(bl[b2], bh[b2]))
            open_bins.remove(fullest)
    return bins


class DirPack:
    """Packing of one direction's dest space for all cores."""

    def __init__(self, dest, src, n, loc, wv, wm_try=24):
        self.n, self.loc, self.wv = n, loc, wv
        order = np.argsort(dest, kind="stable")
        self.dest_s = dest[order]
        self.src_s = src[order]
        counts = np.bincount(dest, minlength=n)
        self.starts = np.concatenate([[0], np.cumsum(counts)])
        # part bit = source local-index parity: part-0 dests go to the first
        # half of the windows (vpos < wv/2), so a half-AllGather of z rows
        # [0:wv/2] is a CONTIGUOUS slab of the part-major gathered table
        self.lo_mask_s = (self.src_s % loc) % 2 == 0

        nwH = wv // 1024              # windows per part-half
        wm = wm_try
        while True:
            wb = 512 // wm
            fits = True
            all_bins = []
            for c in range(NC):
                d0 = c * loc
                lo_cnt = np.zeros(loc, np.int64)
                hi_cnt = np.zeros(loc, np.int64)
                for li in range(loc):
                    s, e = self.starts[d0 + li], self.starts[d0 + li + 1]
                    lo = int(self.lo_mask_s[s:e].sum())
                    lo_cnt[li] = lo
                    hi_cnt[li] = (e - s) - lo
                parts = []
                for part in (0, 1):
                    ids = np.where(np.arange(loc) % 2 == part)[0]
                    pb = _pack_core(lo_cnt[ids], hi_cnt[ids], wm)
                    if len(pb) > nwH * wb:
                        fits = False
                    parts.append([[int(ids[x]) for x in b] for b in pb])
                all_bins.append(parts)
            if fits:
                break
            wm -= 1
            assert wm >= 12, "packing does not fit virtual slice"
        self.wm = wm
        self.wb = 512 // wm           # superbins per psum window
        self.nw = 2 * nwH
        self.nt = self.nw * self.wb   # uniform padded superbin count
        self.width = self.nw * 512    # window-padded: 512 cols per window
        assert self.width <= wv
        # flatten: part-0 bins at [0, nwH*wb), part-1 at [nwH*wb, ...)
        self.core_bins = []
        for parts in all_bins:
            flat = list(parts[0])
            flat += [[] for _ in range(nwH * self.wb - len(parts[0]))]
            flat += parts[1]
            self.core_bins.append(flat)

        self.vpos = np.zeros(n, np.int64)
        for c in range(NC):
            d0 = c * loc
            for b, members in enumerate(self.core_bins[c]):
                w, bl = b // self.wb, b % self.wb
                for j, li in enumerate(members):
                    self.vpos[d0 + li] = w * 512 + bl * wm + j


def build_dir_data(packD: DirPack, packS: DirPack):
    """Per-core device arrays for one direction (packS gives src Z rows)."""
    wm, nt, width = packD.wm, packD.nt, packD.width
    loc, wv = packD.loc, packD.wv
    half = 4 * wv
    src = packD.src_s
    vsrc = packS.vpos[src]
    core = src // loc
    # part-major table: [A: 8 ranks x wv/2][B: 8 ranks x wv/2]
    src_row_s = np.where(
        vsrc < wv // 2,
        core * (wv // 2) + vsrc,
        half + core * (wv // 2) + (vsrc - wv // 2))

    cores = []
    # padding slots get distinct spread-out rows (values are killed by M=0);
    # idx 0 padding makes thousands of descriptors hit one DRAM row, which
    # serializes the DMA engines and collapses late-window gather throughput
    pad_base = (np.arange(nt * 128, dtype=np.int64).reshape(nt, 128)
                % (4 * wv)).astype(np.int16)
    for c in range(NC):
        d0 = c * loc
        idx_lo = pad_base.copy()
        idx_hi = pad_base.copy()
        m_lo = np.zeros((nt, 128, wm), np.float32)
        m_hi = np.zeros((nt, 128, wm), np.float32)
        deg = np.zeros(width, np.float32)
        for b, members in enumerate(packD.core_bins[c]):
            ptr_lo = ptr_hi = 0
            wj, bl = b // packD.wb, b % packD.wb
            for j, li in enumerate(members):
                s, e = packD.starts[d0 + li], packD.starts[d0 + li + 1]
                deg[wj * 512 + bl * wm + j] = float(e - s)
                rows = src_row_s[s:e]
                lo = rows[rows < half]
                hi = rows[rows >= half]
                ur, um = np.unique(lo, return_counts=True)
                k = len(ur)
                idx_lo[b, ptr_lo:ptr_lo + k] = ur
                m_lo[b, ptr_lo:ptr_lo + k, j] = um
                ptr_lo += k
                ur, um = np.unique(hi, return_counts=True)
                k = len(ur)
                idx_hi[b, ptr_hi:ptr_hi + k] = ur - half
                m_hi[b, ptr_hi:ptr_hi + k, j] = um
                ptr_hi += k
            assert ptr_lo <= 128 and ptr_hi <= 128
        cores.append({"idx_lo": idx_lo, "idx_hi": idx_hi,
                      "m_lo": m_lo, "m_hi": m_hi, "deg": deg})
    return cores


def wrap_idx(idx_tiles):
    """[nt, 128] int16 -> SBUF wrapped layout [128, nt*8]."""
    nt = idx_tiles.shape[0]
    out = np.zeros((16, nt * 8), np.int16)
    for t in range(nt):
        out[:, 8 * t:8 * t + 8] = idx_tiles[t].reshape(8, 16).T
    return np.tile(out, (8, 1))


def m_flat(m_tiles):
    """[nt, 128, wm] -> [128, nt*wm] (slot on partitions)."""
    nt, _, wm = m_tiles.shape
    return m_tiles.transpose(1, 0, 2).reshape(128, nt * wm).copy()


def prepare_host_data(inputs, n, loc, wv):
    """All per-core ExternalInput arrays + structural params."""
    import ml_dtypes
    bf = ml_dtypes.bfloat16
    eu = np.asarray(inputs["edge_u"]).astype(np.int64)
    ev = np.asarray(inputs["edge_v"]).astype(np.int64)
    X_v = np.asarray(inputs["X_v"], dtype=np.float32)

    packU = DirPack(eu, ev, n, loc, wv)   # dest u (layers 0, 2)
    packV = DirPack(ev, eu, n, loc, wv)   # dest v (layer 1)
    dataU = build_dir_data(packU, packV)
    dataV = build_dir_data(packV, packU)

    w_all = np.concatenate(
        [np.asarray(inputs[f"W{i}"], np.float32) for i in range(3)], axis=1)
    bias = np.zeros((16, 384), np.float32)
    for i in range(3):
        bias[0, 128 * i:128 * (i + 1)] = np.asarray(inputs[f"b{i}"], np.float32)

    per_core = []
    for c in range(NC):
        xT = np.zeros((128, packV.width), np.float32)
        g = np.arange(c * loc, (c + 1) * loc)
        xT[:, packV.vpos[g]] = X_v[g].T
        degu = np.zeros((16, packU.width), np.float32)
        degu[0] = dataU[c]["deg"]
        degv = np.zeros((16, packV.width), np.float32)
        degv[0] = dataV[c]["deg"]
        per_core.append({
            "xT": xT.astype(bf),
            "w_all": w_all.astype(bf),
            "bias": bias.astype(bf),
            "degu": degu.astype(bf),
            "degv": degv.astype(bf),
            "idxu_lo": wrap_idx(dataU[c]["idx_lo"]),
            "idxu_hi": wrap_idx(dataU[c]["idx_hi"]),
            "idxv_lo": wrap_idx(dataV[c]["idx_lo"]),
            "idxv_hi": wrap_idx(dataV[c]["idx_hi"]),
            "mu_lo": m_flat(dataU[c]["m_lo"]).astype(bf),
            "mu_hi": m_flat(dataU[c]["m_hi"]).astype(bf),
            "mv_lo": m_flat(dataV[c]["m_lo"]).astype(bf),
            "mv_hi": m_flat(dataV[c]["m_hi"]).astype(bf),
        })
    return packU, packV, per_core


# ----------------------------------------------------------------------------
# walrus drain workaround: split multi-wait tail Drain into single-wait nops
# ----------------------------------------------------------------------------


def _patch_tile_drain():
    from concourse import tile
    if getattr(tile.TileContext, "_bgnn_drain_patched", False):
        return
    from concourse.vector_clock import ScopedClock

    def patched(self, tick_clock, wait_clock):
        nc = self.nc
        nops = [nc.sync.nop() for _ in range(31)]
        drain_inst = nc.sync.drain()
        wait_clock.add_sem_waits(
            drain_inst.ins, ScopedClock({None: tick_clock.global_clock})
        )
        si = drain_inst.ins.sync_info
        waits = list(si.on_wait) if si is not None else []
        if len(waits) > 1:
            assert len(waits) - 1 <= len(nops), len(waits)
            for i, w in enumerate(waits[:-1]):
                n = nops[i].ins
                nsi = n.sync_info
                if nsi is None:
                    n.sync_info = type(si)(on_wait=[w], on_update=[])
                else:
                    nsi.on_wait = list(nsi.on_wait) + [w]
            si.on_wait = waits[-1:]
        nc.all_engine_barrier()
        popped = nc._tile_sem_poison_stack.pop()
        assert popped is self._sem_poison
        nc.clear_and_free_semaphores(list(self.sems.allocated().values()))
        nc.all_engine_barrier()

    tile.TileContext._drain_and_barrier = patched
    tile.TileContext._bgnn_drain_patched = True


# ----------------------------------------------------------------------------
# device program
# ----------------------------------------------------------------------------


def build_program(packU: DirPack, packV: DirPack, max_steps: int = 99):
    """max_steps: debug truncation. Each layer = 3 steps (dense, AG, agg)."""
    import concourse.bass as bass
    import concourse.mybir as mybir
    from concourse import bacc, tile

    _patch_tile_drain()
    f32 = mybir.dt.float32
    bf16 = mybir.dt.bfloat16
    i16 = mybir.dt.int16

    wv = packU.wv
    zrows = NC * wv
    half = 4 * wv
    widthU, widthV = packU.width, packV.width
    wmax = max(widthU, widthV)

    nc = bacc.Bacc(num_swdge_queues=4)
    core_ids = list(range(NC))

    # I/O
    xT_d = nc.dram_tensor("xT", [128, widthV], bf16, kind="ExternalInput")
    w_d = nc.dram_tensor("w_all", [128, 384], bf16, kind="ExternalInput")
    bias_d = nc.dram_tensor("bias", [16, 384], bf16, kind="ExternalInput")
    degu_d = nc.dram_tensor("degu", [16, widthU], bf16, kind="ExternalInput")
    degv_d = nc.dram_tensor("degv", [16, widthV], bf16, kind="ExternalInput")
    idx_d = {}
    m_d = {}
    for dirn, pk in (("u", packU), ("v", packV)):
        for s in ("lo", "hi"):
            idx_d[dirn, s] = nc.dram_tensor(
                f"idx{dirn}_{s}", [128, pk.nt * 8], i16, kind="ExternalInput")
            m_d[dirn, s] = nc.dram_tensor(
                f"m{dirn}_{s}", [128, pk.nt * pk.wm], bf16, kind="ExternalInput")
    out_d = nc.dram_tensor("outp", [128, widthU], bf16, kind="ExternalOutput")

    # internal DRAM: z halves + ping-pong per-part gathered tables.
    # Separate A/B tensors give exact deps: the lo-stream gathers (A-part
    # sources) only wait for AGa, so they run WHILE AGb still transfers.
    z_a = nc.dram_tensor("z_a", [wv // 2, 128], bf16)
    z_b = nc.dram_tensor("z_b", [wv // 2, 128], bf16)
    ztabs = [
        [nc.dram_tensor(f"z_t{t}{pp}", [half, 128], bf16, addr_space="Shared")
         for pp in ("A", "B")]
        for t in range(2)
    ]

    layers = [
        ("u", packU, degu_d, 0),   # layer 0: dense over V layout, agg to U
        ("v", packV, degv_d, 1),   # layer 1: dense over U layout, agg to V
        ("u", packU, degu_d, 2),   # layer 2: dense over V layout, agg to U
    ]

    nwU, nwV = packU.nw, packV.nw

    with tile.TileContext(nc) as tc:
        with (
            tc.tile_pool(name="persist", bufs=1) as persist,
            tc.tile_pool(name="zstage", bufs=2) as zstage_pool,
            tc.tile_pool(name="gchunk", bufs=26) as g_pool,
            tc.tile_pool(name="mslab", bufs=8) as m_pool,
            tc.tile_pool(name="degslab", bufs=4) as deg_pool,
            tc.tile_pool(name="pagg", bufs=3, space="PSUM") as pagg_pool,
            tc.tile_pool(name="pdense", bufs=4, space="PSUM") as pdense_pool,
        ):
            # per-window input/agg-output tiles: inA = V layout, inB = U layout
            inA_t = [persist.tile([128, 512], bf16, tag=f"inA{w}", name=f"inA{w}")
                     for w in range(nwV)]
            inB_t = [persist.tile([128, 512], bf16, tag=f"inB{w}", name=f"inB{w}")
                     for w in range(nwU)]
            w_sb = persist.tile([128, 384], bf16, tag="w")
            bias_sb = persist.tile([16, 384], bf16, tag="bias")
            idx_sb = {}
            for dirn, pk in (("u", packU), ("v", packV)):
                for s in ("lo", "hi"):
                    idx_sb[dirn, s] = persist.tile(
                        [128, pk.nt * 8], i16, tag=f"idx{dirn}{s}",
                        name=f"idx{dirn}{s}")

            # zero the dead window-tile columns (wcols:512): they are never
            # written, but dense multiplies them into z rows that padding
            # gathers may fetch, and 0 * garbage-NaN = NaN in PSUM
            for pk_, tiles in ((packU, inB_t), (packV, inA_t)):
                wc = pk_.wb * pk_.wm
                if wc < 512:
                    for t in tiles:
                        nc.vector.memset(t[:, wc:512], 0.0)

            # preload
            for w in range(nwV):
                nc.sync.dma_start(out=inA_t[w][:],
                                  in_=xT_d[:, w * 512:(w + 1) * 512])
            nc.sync.dma_start(out=w_sb[:], in_=w_d[:])
            nc.sync.dma_start(out=bias_sb[:], in_=bias_d[:])
            for key, t in idx_sb.items():
                nc.sync.dma_start(out=t[:], in_=idx_d[key][:])

            gather_regs = {}
            gather_call_no = [0]
            prep_sems = [nc.alloc_semaphore(name=f"prepsem{q}")
                         for q in range(4)]
            CH = 8                  # gather tiles per call (1024 idxs)
            PREP_W = 0              # windows desc-prepped during the AG

            def emit_dense_stage(cur_in, wl, stg, nwS):
                """z[k0*128:(k0+hc)*128, :] = in.T @ W_wl for one half."""
                hc = nwS * 2
                k0, k1 = stg * hc, (stg + 1) * hc
                ztgt = z_a if stg == 0 else z_b
                zst = zstage_pool.tile([128, hc * 128], bf16, tag="zst")
                for k in range(k0, k1):
                    pz = pdense_pool.tile([128, 128], f32, tag="pz")
                    nc.tensor.matmul(
                        pz[:],
                        lhsT=cur_in[k // 4][:, (k % 4) * 128:(k % 4 + 1) * 128],
                        rhs=w_sb[:, wl * 128:(wl + 1) * 128],
                        start=True, stop=True,
                    )
                    nc.vector.tensor_copy(
                        zst[:, (k - k0) * 128:(k - k0) * 128 + 128], pz[:])
                # SBUF [p, k, f] -> DRAM rows (k-k0)*128+p of the half
                src_ap = zst[:].rearrange("p (k f) -> p k f", f=128)
                dst_ap = ztgt.rearrange(
                    "(kk p) f -> p kk f", p=128)[:, 0:hc, :]
                nc.sync.dma_start(out=dst_ap, in_=src_ap)

            def emit_ag(li2, part):
                """Half-AllGather of z_a/z_b into layer li2's ping-pong
                table; part-major layout keeps both outputs contiguous."""
                zin = z_a if part == 0 else z_b
                nc.gpsimd.collective_compute(
                    "AllGather",
                    mybir.AluOpType.bypass,
                    replica_groups=[core_ids],
                    ins=[zin[:]],
                    outs=[ztabs[li2 % 2][part][:]],
                )

            def emit_gather(dirn, pk, w, stream, ci, prep):
                wb = pk.wb
                c0, c1 = ci * CH, min(wb, ci * CH + CH)
                nci = (c1 - c0) * 128
                if nci not in gather_regs:
                    gather_regs[nci] = nc.gpsimd.to_reg(nci)
                gbuf = g_pool.tile([128, nci], bf16, tag="g")
                src = (zfull_d[0:half, :] if stream == "lo"
                       else zfull_d[half:2 * half, :])
                q = gather_call_no[0] % 4
                kw = dict(prepare_only=True, sem=prep_sems[q]) if prep else {}
                nc.gpsimd.dma_gather(
                    gbuf[:].rearrange("p (t e) -> p t e", e=128),
                    src,
                    idx_sb[dirn, stream][
                        :, (w * wb + c0) * 8:(w * wb + c1) * 8],
                    num_idxs=nci,
                    num_idxs_reg=gather_regs[nci],
                    elem_size=128,
                    queue_num=q,
                    **kw,
                )
                gather_call_no[0] += 1
                return gbuf

            bufs = [inA_t, inB_t]
            # layer-0 dense feeds off the preloaded X_v tiles; its first
            # half-AG launches while the second dense half still runs
            emit_dense_stage(inA_t, 0, 0, nwV)
            emit_ag(0, 0)
            emit_dense_stage(inA_t, 0, 1, nwV)

            for li, (dirn, pk, deg_d, wl) in enumerate(layers):
                cur_in = bufs[li % 2]
                dst = bufs[(li + 1) % 2]
                last = li + 1 >= len(layers)

                # second half-AG (the first was emitted mid-previous-agg and
                # overlapped the remaining gather windows there)
                emit_ag(li, 1)
                tabA, tabB = ztabs[li % 2]

                wm, wb, nw = pk.wm, pk.wb, pk.nw
                nt_pk = pk.nt
                wcols = wb * wm

                # gather calls are CH-tile chunks of the per-stream global
                # tile sequence, independent of window boundaries, so every
                # call carries a full CH*128 descriptors
                g_chunks = {"lo": [], "hi": []}
                emitted = {"lo": 0, "hi": 0}

                LEAD = 64           # lo-tiles emitted ahead of hi (8 calls)

                def pump_gathers(upto):
                    tgt = {"lo": min(upto + LEAD, nt_pk),
                           "hi": min(upto, nt_pk)}
                    while emitted["lo"] < tgt["lo"] or emitted["hi"] < tgt["hi"]:
                        for stream in ("lo", "hi"):
                            c0 = emitted[stream]
                            if c0 >= tgt[stream]:
                                continue
                            c1 = min(c0 + CH, nt_pk)
                            nci = (c1 - c0) * 128
                            if nci not in gather_regs:
                                gather_regs[nci] = nc.gpsimd.to_reg(nci)
                            gbuf = g_pool.tile([128, nci], bf16, tag="g")
                            src = (tabA[:] if stream == "lo" else tabB[:])
                            nc.gpsimd.dma_gather(
                                gbuf[:].rearrange("p (t e) -> p t e", e=128),
                                src,
                                idx_sb[dirn, stream][:, c0 * 8:c1 * 8],
                                num_idxs=nci,
                                num_idxs_reg=gather_regs[nci],
                                elem_size=128,
                                queue_num=gather_call_no[0] % 4,
                            )
                            gather_call_no[0] += 1
                            g_chunks[stream].append(gbuf)
                            emitted[stream] = c1

                for w in range(nw):
                    pump_gathers((w + 1) * wb)
                    m_lo = m_pool.tile([128, wcols], bf16, tag="mlo")
                    m_hi = m_pool.tile([128, wcols], bf16, tag="mhi")
                    nc.sync.dma_start(
                        out=m_lo[:], in_=m_d[dirn, "lo"][:, w * wcols:(w + 1) * wcols])
                    nc.sync.dma_start(
                        out=m_hi[:], in_=m_d[dirn, "hi"][:, w * wcols:(w + 1) * wcols])
                    deg_sl = deg_pool.tile([16, wcols], bf16, tag="deg")
                    nc.sync.dma_start(
                        out=deg_sl[:], in_=deg_d[0:16, w * 512:w * 512 + wcols])

                    pw = pagg_pool.tile([128, wcols], f32, tag="pagg")
                    nc.tensor.matmul(
                        pw[:],
                        lhsT=bias_sb[0:16, wl * 128:(wl + 1) * 128],
                        rhs=deg_sl[:],
                        start=True, stop=False, skip_group_check=True,
                    )
                    for stream, m_sl in (("lo", m_lo), ("hi", m_hi)):
                        for t in range(wb):
                            gt = w * wb + t
                            nc.tensor.matmul(
                                pw[:, t * wm:(t + 1) * wm],
                                lhsT=g_chunks[stream][gt // CH][
                                    :, (gt % CH) * 128:(gt % CH + 1) * 128],
                                rhs=m_sl[:, t * wm:(t + 1) * wm],
                                start=False,
                                stop=(stream == "hi" and t == wb - 1),
                                skip_group_check=True,
                            )
                    nc.vector.tensor_copy(dst[w][:, 0:wcols], pw[:])
                    if last:
                        # drain output windows during the remaining agg
                        nc.sync.dma_start(
                            out=out_d[:, w * 512:(w + 1) * 512], in_=dst[w][:])
                    elif w == nw // 2 - 1:
                        # next layer's dense first half only needs windows
                        # 0..nw/2-1; its half-AG then overlaps the remaining
                        # gather windows (it writes the OTHER ping-pong table)
                        emit_dense_stage(dst, layers[li + 1][3], 0, nw)
                        emit_ag(li + 1, 0)
                if not last:
                    emit_dense_stage(dst, layers[li + 1][3], 1, nw)

    nc.compile()
    return nc


# ----------------------------------------------------------------------------
# public entry point
# ----------------------------------------------------------------------------


def kernel(**inputs) -> np.ndarray:
    from concourse.bass_utils import run_bass_kernel_spmd

    n = int(np.asarray(inputs["X_u"]).shape[0])
    loc = n // NC
    # wv: virtual rows per core slice; 4*wv must be >= any lo/hi idx range
    wv = 8192 if n == 50000 else max(512, 1 << (loc * 2 - 1).bit_length())

    packU, packV, per_core = prepare_host_data(inputs, n, loc, wv)
    nc = build_program(packU, packV)
    res = run_bass_kernel_spmd(nc, per_core, list(range(NC)))

    out = np.zeros((n, 128), np.float32)
    for c in range(NC):
        g = np.arange(c * loc, (c + 1) * loc)
        out[g] = res.results[c]["outp"][:, packU.vpos[g]].T.astype(np.float32)
    return out


if __name__ == "__main__":
    data = dict(np.load("/root/problem/inputs_cache.npz"))
    got = kernel(**data)
    np.save("/root/problem/kernel_out.npy", got)
    print("kernel done", got.shape)



# revision 24
# speedup vs baseline: 1.0210x; 1.0210x over previous
"""Trainium2 Bass kernel for nn_BGNN_MLP (bipartite 3-layer GNN).

Self-contained: kernel(**inputs) -> np.ndarray takes the full unsharded
inputs and returns the full [50000, 128] output, running on 8 NeuronCores
via run_bass_kernel_spmd.

Algorithm (per layer l = 0,1,2; directions U,V,U):
  z = input @ W_l            (dense, per-core slice, node-major)
  publish z slice -> AllGather -> Z table [8*WV, 128] in DRAM
  aggregate: out[d] = sum_{edges e: dest(e)=d} z[src(e)]  + deg(d)*b_l
    via per-superbin gather tiles (dma_gather, 128 edge slots/tile) and
    PE matmuls (gathered rows stationary, 0/1 selector M moving) into
    PSUM windows; the bias enters as a rank-1 outer(b, deg) matmul that
    also initializes each window.

SPMD: one instruction stream for all 8 cores; all per-core variation is
carried by ExternalInput data (packing layout, gather indices, M, deg).
"""

import sys

if "/opt/trn_rl_repo" not in sys.path:
    sys.path.insert(0, "/opt/trn_rl_repo")

import numpy as np

NC = 8

# ----------------------------------------------------------------------------
# host-side packing
# ----------------------------------------------------------------------------


def _pack_core(lo_cnt, hi_cnt, wm):
    """2D FFD, imbalance-aware. Returns list of bins (lists of local ids)."""
    order = np.argsort(-(lo_cnt + hi_cnt), kind="stable")
    bins, bl, bh = [], [], []
    open_bins = []
    for li in order:
        li = int(li)
        l, h = int(lo_cnt[li]), int(hi_cnt[li])
        best, best_score = -1, None
        for bi in open_bins:
            if len(bins[bi]) >= wm:
                continue
            nl, nh = bl[bi] + l, bh[bi] + h
            if nl > 128 or nh > 128:
                continue
            score = abs(nl - nh)
            if best_score is None or score < best_score:
                best_score, best = score, bi
        if best < 0:
            bins.append([li]); bl.append(l); bh.append(h)
        else:
            bins[best].append(li); bl[best] += l; bh[best] += h
        bi = best if best >= 0 else len(bins) - 1
        if bi not in open_bins:
            if not (max(bl[bi], bh[bi]) > 127 or len(bins[bi]) >= wm):
                open_bins.append(bi)
        elif max(bl[bi], bh[bi]) > 127 or len(bins[bi]) >= wm:
            open_bins.remove(bi)
        if len(open_bins) > 48:
            fullest = max(open_bins, key=lambda b2: max(bl[b2], bh[b2]))
            open_bins.remove(fullest)
    return bins


class DirPack:
    """Packing of one direction's dest space for all cores."""

    def __init__(self, dest, src, n, loc, wv, wm_try=24):
        self.n, self.loc, self.wv = n, loc, wv
        order = np.argsort(dest, kind="stable")
        self.dest_s = dest[order]
        self.src_s = src[order]
        counts = np.bincount(dest, minlength=n)
        self.starts = np.concatenate([[0], np.cumsum(counts)])
        # part bit = source local-index parity: part-0 dests go to the first
        # half of the windows (vpos < wv/2), so a half-AllGather of z rows
        # [0:wv/2] is a CONTIGUOUS slab of the part-major gathered table
        self.lo_mask_s = (self.src_s % loc) % 2 == 0

        nwH = wv // 1024              # windows per part-half
        wm = wm_try
        while True:
            wb = 512 // wm
            fits = True
            all_bins = []
            for c in range(NC):
                d0 = c * loc
                lo_cnt = np.zeros(loc, np.int64)
                hi_cnt = np.zeros(loc, np.int64)
                for li in range(loc):
                    s, e = self.starts[d0 + li], self.starts[d0 + li + 1]
                    lo = int(self.lo_mask_s[s:e].sum())
                    lo_cnt[li] = lo
                    hi_cnt[li] = (e - s) - lo
                parts = []
                for part in (0, 1):
                    ids = np.where(np.arange(loc) % 2 == part)[0]
                    pb = _pack_core(lo_cnt[ids], hi_cnt[ids], wm)
                    if len(pb) > nwH * wb:
                        fits = False
                    parts.append([[int(ids[x]) for x in b] for b in pb])
                all_bins.append(parts)
            if fits:
                break
            wm -= 1
            assert wm >= 12, "packing does not fit virtual slice"
        self.wm = wm
        self.wb = 512 // wm           # superbins per psum window
        self.nw = 2 * nwH
        self.nt = self.nw * self.wb   # uniform padded superbin count
        self.width = self.nw * 512    # window-padded: 512 cols per window
        assert self.width <= wv
        # flatten: part-0 bins at [0, nwH*wb), part-1 at [nwH*wb, ...)
        self.core_bins = []
        for parts in all_bins:
            flat = list(parts[0])
            flat += [[] for _ in range(nwH * self.wb - len(parts[0]))]
            flat += parts[1]
            self.core_bins.append(flat)

        self.vpos = np.zeros(n, np.int64)
        for c in range(NC):
            d0 = c * loc
            for b, members in enumerate(self.core_bins[c]):
                w, bl = b // self.wb, b % self.wb
                for j, li in enumerate(members):
                    self.vpos[d0 + li] = w * 512 + bl * wm + j


def build_dir_data(packD: DirPack, packS: DirPack):
    """Per-core device arrays for one direction (packS gives src Z rows)."""
    wm, nt, width = packD.wm, packD.nt, packD.width
    loc, wv = packD.loc, packD.wv
    half = 4 * wv
    src = packD.src_s
    vsrc = packS.vpos[src]
    core = src // loc
    # part-major table: [A: 8 ranks x wv/2][B: 8 ranks x wv/2]
    src_row_s = np.where(
        vsrc < wv // 2,
        core * (wv // 2) + vsrc,
        half + core * (wv // 2) + (vsrc - wv // 2))

    cores = []
    # padding slots get distinct spread-out rows (values are killed by M=0);
    # idx 0 padding makes thousands of descriptors hit one DRAM row, which
    # serializes the DMA engines and collapses late-window gather throughput
    pad_base = (np.arange(nt * 128, dtype=np.int64).reshape(nt, 128)
                % (4 * wv)).astype(np.int16)
    for c in range(NC):
        d0 = c * loc
        idx_lo = pad_base.copy()
        idx_hi = pad_base.copy()
        m_lo = np.zeros((nt, 128, wm), np.float32)
        m_hi = np.zeros((nt, 128, wm), np.float32)
        deg = np.zeros(width, np.float32)
        for b, members in enumerate(packD.core_bins[c]):
            ptr_lo = ptr_hi = 0
            wj, bl = b // packD.wb, b % packD.wb
            for j, li in enumerate(members):
                s, e = packD.starts[d0 + li], packD.starts[d0 + li + 1]
                deg[wj * 512 + bl * wm + j] = float(e - s)
                rows = src_row_s[s:e]
                lo = rows[rows < half]
                hi = rows[rows >= half]
                ur, um = np.unique(lo, return_counts=True)
                k = len(ur)
                idx_lo[b, ptr_lo:ptr_lo + k] = ur
                m_lo[b, ptr_lo:ptr_lo + k, j] = um
                ptr_lo += k
                ur, um = np.unique(hi, return_counts=True)
                k = len(ur)
                idx_hi[b, ptr_hi:ptr_hi + k] = ur - half
                m_hi[b, ptr_hi:ptr_hi + k, j] = um
                ptr_hi += k
            assert ptr_lo <= 128 and ptr_hi <= 128
        cores.append({"idx_lo": idx_lo, "idx_hi": idx_hi,
                      "m_lo": m_lo, "m_hi": m_hi, "deg": deg})
    return cores


def wrap_idx(idx_tiles):
    """[nt, 128] int16 -> SBUF wrapped layout [128, nt*8]."""
    nt = idx_tiles.shape[0]
    out = np.zeros((16, nt * 8), np.int16)
    for t in range(nt):
        out[:, 8 * t:8 * t + 8] = idx_tiles[t].reshape(8, 16).T
    return np.tile(out, (8, 1))


def m_flat(m_tiles):
    """[nt, 128, wm] -> [128, nt*wm] (slot on partitions)."""
    nt, _, wm = m_tiles.shape
    return m_tiles.transpose(1, 0, 2).reshape(128, nt * wm).copy()


def prepare_host_data(inputs, n, loc, wv):
    """All per-core ExternalInput arrays + structural params."""
    import ml_dtypes
    bf = ml_dtypes.bfloat16
    eu = np.asarray(inputs["edge_u"]).astype(np.int64)
    ev = np.asarray(inputs["edge_v"]).astype(np.int64)
    X_v = np.asarray(inputs["X_v"], dtype=np.float32)

    packU = DirPack(eu, ev, n, loc, wv)   # dest u (layers 0, 2)
    packV = DirPack(ev, eu, n, loc, wv)   # dest v (layer 1)
    dataU = build_dir_data(packU, packV)
    dataV = build_dir_data(packV, packU)

    w_all = np.concatenate(
        [np.asarray(inputs[f"W{i}"], np.float32) for i in range(3)], axis=1)
    bias = np.zeros((16, 384), np.float32)
    for i in range(3):
        bias[0, 128 * i:128 * (i + 1)] = np.asarray(inputs[f"b{i}"], np.float32)

    per_core = []
    for c in range(NC):
        xT = np.zeros((128, packV.width), np.float32)
        g = np.arange(c * loc, (c + 1) * loc)
        xT[:, packV.vpos[g]] = X_v[g].T
        degu = np.zeros((16, packU.width), np.float32)
        degu[0] = dataU[c]["deg"]
        degv = np.zeros((16, packV.width), np.float32)
        degv[0] = dataV[c]["deg"]
        per_core.append({
            "xT": xT.astype(bf),
            "w_all": w_all.astype(bf),
            "bias": bias.astype(bf),
            "degu": degu.astype(bf),
            "degv": degv.astype(bf),
            "idxu_lo": wrap_idx(dataU[c]["idx_lo"]),
            "idxu_hi": wrap_idx(dataU[c]["idx_hi"]),
            "idxv_lo": wrap_idx(dataV[c]["idx_lo"]),
            "idxv_hi": wrap_idx(dataV[c]["idx_hi"]),
            "mu_lo": m_flat(dataU[c]["m_lo"]).astype(bf),
            "mu_hi": m_flat(dataU[c]["m_hi"]).astype(bf),
            "mv_lo": m_flat(dataV[c]["m_lo"]).astype(bf),
            "mv_hi": m_flat(dataV[c]["m_hi"]).astype(bf),
        })
    return packU, packV, per_core


# ----------------------------------------------------------------------------
# walrus drain workaround: split multi-wait tail Drain into single-wait nops
# ----------------------------------------------------------------------------


def _patch_tile_drain():
    from concourse import tile
    if getattr(tile.TileContext, "_bgnn_drain_patched", False):
        return
    from concourse.vector_clock import ScopedClock

    def patched(self, tick_clock, wait_clock):
        nc = self.nc
        nops = [nc.sync.nop() for _ in range(31)]
        drain_inst = nc.sync.drain()
        wait_clock.add_sem_waits(
            drain_inst.ins, ScopedClock({None: tick_clock.global_clock})
        )
        si = drain_inst.ins.sync_info
        waits = list(si.on_wait) if si is not None else []
        if len(waits) > 1:
            assert len(waits) - 1 <= len(nops), len(waits)
            for i, w in enumerate(waits[:-1]):
                n = nops[i].ins
                nsi = n.sync_info
                if nsi is None:
                    n.sync_info = type(si)(on_wait=[w], on_update=[])
                else:
                    nsi.on_wait = list(nsi.on_wait) + [w]
            si.on_wait = waits[-1:]
        nc.all_engine_barrier()
        popped = nc._tile_sem_poison_stack.pop()
        assert popped is self._sem_poison
        nc.clear_and_free_semaphores(list(self.sems.allocated().values()))
        nc.all_engine_barrier()

    tile.TileContext._drain_and_barrier = patched
    tile.TileContext._bgnn_drain_patched = True


# ----------------------------------------------------------------------------
# device program
# ----------------------------------------------------------------------------


def build_program(packU: DirPack, packV: DirPack, max_steps: int = 99):
    """max_steps: debug truncation. Each layer = 3 steps (dense, AG, agg)."""
    import concourse.bass as bass
    import concourse.mybir as mybir
    from concourse import bacc, tile

    _patch_tile_drain()
    f32 = mybir.dt.float32
    bf16 = mybir.dt.bfloat16
    i16 = mybir.dt.int16

    wv = packU.wv
    zrows = NC * wv
    half = 4 * wv
    widthU, widthV = packU.width, packV.width
    wmax = max(widthU, widthV)

    nc = bacc.Bacc(num_swdge_queues=4)
    core_ids = list(range(NC))

    # I/O
    xT_d = nc.dram_tensor("xT", [128, widthV], bf16, kind="ExternalInput")
    w_d = nc.dram_tensor("w_all", [128, 384], bf16, kind="ExternalInput")
    bias_d = nc.dram_tensor("bias", [16, 384], bf16, kind="ExternalInput")
    degu_d = nc.dram_tensor("degu", [16, widthU], bf16, kind="ExternalInput")
    degv_d = nc.dram_tensor("degv", [16, widthV], bf16, kind="ExternalInput")
    idx_d = {}
    m_d = {}
    for dirn, pk in (("u", packU), ("v", packV)):
        for s in ("lo", "hi"):
            idx_d[dirn, s] = nc.dram_tensor(
                f"idx{dirn}_{s}", [128, pk.nt * 8], i16, kind="ExternalInput")
            m_d[dirn, s] = nc.dram_tensor(
                f"m{dirn}_{s}", [128, pk.nt * pk.wm], bf16, kind="ExternalInput")
    out_d = nc.dram_tensor("outp", [128, widthU], bf16, kind="ExternalOutput")

    # internal DRAM: z halves + ping-pong part-major gathered tables
    z_a = nc.dram_tensor("z_a", [wv // 2, 128], bf16)
    z_b = nc.dram_tensor("z_b", [wv // 2, 128], bf16)
    ztabs = [
        nc.dram_tensor("z_fullA", [zrows, 128], bf16, addr_space="Shared"),
        nc.dram_tensor("z_fullB", [zrows, 128], bf16, addr_space="Shared"),
    ]

    layers = [
        ("u", packU, degu_d, 0),   # layer 0: dense over V layout, agg to U
        ("v", packV, degv_d, 1),   # layer 1: dense over U layout, agg to V
        ("u", packU, degu_d, 2),   # layer 2: dense over V layout, agg to U
    ]

    nwU, nwV = packU.nw, packV.nw

    with tile.TileContext(nc) as tc:
        with (
            tc.tile_pool(name="persist", bufs=1) as persist,
            tc.tile_pool(name="zstage", bufs=2) as zstage_pool,
            tc.tile_pool(name="gchunk", bufs=16) as g_pool,
            tc.tile_pool(name="mslab", bufs=8) as m_pool,
            tc.tile_pool(name="degslab", bufs=4) as deg_pool,
            tc.tile_pool(name="pagg", bufs=3, space="PSUM") as pagg_pool,
            tc.tile_pool(name="pdense", bufs=4, space="PSUM") as pdense_pool,
        ):
            # per-window input/agg-output tiles: inA = V layout, inB = U layout
            inA_t = [persist.tile([128, 512], bf16, tag=f"inA{w}", name=f"inA{w}")
                     for w in range(nwV)]
            inB_t = [persist.tile([128, 512], bf16, tag=f"inB{w}", name=f"inB{w}")
                     for w in range(nwU)]
            w_sb = persist.tile([128, 384], bf16, tag="w")
            bias_sb = persist.tile([16, 384], bf16, tag="bias")
            idx_sb = {}
            for dirn, pk in (("u", packU), ("v", packV)):
                for s in ("lo", "hi"):
                    idx_sb[dirn, s] = persist.tile(
                        [128, pk.nt * 8], i16, tag=f"idx{dirn}{s}",
                        name=f"idx{dirn}{s}")

            # zero the dead window-tile columns (wcols:512): they are never
            # written, but dense multiplies them into z rows that padding
            # gathers may fetch, and 0 * garbage-NaN = NaN in PSUM
            for pk_, tiles in ((packU, inB_t), (packV, inA_t)):
                wc = pk_.wb * pk_.wm
                if wc < 512:
                    for t in tiles:
                        nc.vector.memset(t[:, wc:512], 0.0)

            # preload; dense0 stage0 gates AGa(0), so its inputs (weights +
            # first xT half) go first on the sync queue, and everything not
            # needed until the aggregation moves to the vector engine's queue
            nc.sync.dma_start(out=w_sb[:], in_=w_d[:])
            nc.sync.dma_start(out=bias_sb[:], in_=bias_d[:])
            for w in range(nwV // 2):
                nc.sync.dma_start(out=inA_t[w][:],
                                  in_=xT_d[:, w * 512:(w + 1) * 512])
            for w in range(nwV // 2, nwV):
                nc.scalar.dma_start(out=inA_t[w][:],
                                    in_=xT_d[:, w * 512:(w + 1) * 512])
            for key, t in idx_sb.items():
                nc.scalar.dma_start(out=t[:], in_=idx_d[key][:])

            gather_regs = {}
            gather_call_no = [0]
            prep_sems = [nc.alloc_semaphore(name=f"prepsem{q}")
                         for q in range(4)]
            CH = 8                  # gather tiles per call (1024 idxs)
            PREP_W = 0              # windows desc-prepped during the AG

            def emit_dense_stage(cur_in, wl, stg, nwS):
                """z[k0*128:(k0+hc)*128, :] = in.T @ W_wl for one half."""
                hc = nwS * 2
                k0, k1 = stg * hc, (stg + 1) * hc
                ztgt = z_a if stg == 0 else z_b
                zst = zstage_pool.tile([128, hc * 128], bf16, tag="zst")
                for k in range(k0, k1):
                    pz = pdense_pool.tile([128, 128], f32, tag="pz")
                    nc.tensor.matmul(
                        pz[:],
                        lhsT=cur_in[k // 4][:, (k % 4) * 128:(k % 4 + 1) * 128],
                        rhs=w_sb[:, wl * 128:(wl + 1) * 128],
                        start=True, stop=True,
                    )
                    nc.vector.tensor_copy(
                        zst[:, (k - k0) * 128:(k - k0) * 128 + 128], pz[:])
                # SBUF [p, k, f] -> DRAM rows (k-k0)*128+p of the half
                src_ap = zst[:].rearrange("p (k f) -> p k f", f=128)
                dst_ap = ztgt.rearrange(
                    "(kk p) f -> p kk f", p=128)[:, 0:hc, :]
                nc.sync.dma_start(out=dst_ap, in_=src_ap)

            def emit_ag(li2, part):
                """Half-AllGather of z_a/z_b into layer li2's ping-pong
                table; part-major layout keeps both outputs contiguous."""
                zin = z_a if part == 0 else z_b
                tab = ztabs[li2 % 2]
                outs = tab[0:half, :] if part == 0 else tab[half:2 * half, :]
                nc.gpsimd.collective_compute(
                    "AllGather",
                    mybir.AluOpType.bypass,
                    replica_groups=[core_ids],
                    ins=[zin[:]],
                    outs=[outs],
                )

            def emit_gather(dirn, pk, w, stream, ci, prep):
                wb = pk.wb
                c0, c1 = ci * CH, min(wb, ci * CH + CH)
                nci = (c1 - c0) * 128
                if nci not in gather_regs:
                    gather_regs[nci] = nc.gpsimd.to_reg(nci)
                gbuf = g_pool.tile([128, nci], bf16, tag="g")
                src = (zfull_d[0:half, :] if stream == "lo"
                       else zfull_d[half:2 * half, :])
                q = gather_call_no[0] % 4
                kw = dict(prepare_only=True, sem=prep_sems[q]) if prep else {}
                nc.gpsimd.dma_gather(
                    gbuf[:].rearrange("p (t e) -> p t e", e=128),
                    src,
                    idx_sb[dirn, stream][
                        :, (w * wb + c0) * 8:(w * wb + c1) * 8],
                    num_idxs=nci,
                    num_idxs_reg=gather_regs[nci],
                    elem_size=128,
                    queue_num=q,
                    **kw,
                )
                gather_call_no[0] += 1
                return gbuf

            bufs = [inA_t, inB_t]
            # layer-0 dense feeds off the preloaded X_v tiles; its first
            # half-AG launches while the second dense half still runs
            emit_dense_stage(inA_t, 0, 0, nwV)
            emit_ag(0, 0)
            emit_dense_stage(inA_t, 0, 1, nwV)

            for li, (dirn, pk, deg_d, wl) in enumerate(layers):
                cur_in = bufs[li % 2]
                dst = bufs[(li + 1) % 2]
                last = li + 1 >= len(layers)

                # second half-AG (the first was emitted mid-previous-agg and
                # overlapped the remaining gather windows there)
                emit_ag(li, 1)
                zfull_d = ztabs[li % 2]

                wm, wb, nw = pk.wm, pk.wb, pk.nw
                nt_pk = pk.nt
                wcols = wb * wm

                # gather calls are CH-tile chunks of the per-stream global
                # tile sequence, independent of window boundaries, so every
                # call carries a full CH*128 descriptors
                g_chunks = {"lo": [], "hi": []}
                emitted = {"lo": 0, "hi": 0}

                def pump_gathers(upto):
                    upto = min(upto, nt_pk)
                    while emitted["lo"] < upto or emitted["hi"] < upto:
                        for stream in ("lo", "hi"):
                            c0 = emitted[stream]
                            if c0 >= upto:
                                continue
                            c1 = min(c0 + CH, nt_pk)
                            nci = (c1 - c0) * 128
                            if nci not in gather_regs:
                                gather_regs[nci] = nc.gpsimd.to_reg(nci)
                            gbuf = g_pool.tile([128, nci], bf16, tag="g")
                            src = (zfull_d[0:half, :] if stream == "lo"
                                   else zfull_d[half:2 * half, :])
                            nc.gpsimd.dma_gather(
                                gbuf[:].rearrange("p (t e) -> p t e", e=128),
                                src,
                                idx_sb[dirn, stream][:, c0 * 8:c1 * 8],
                                num_idxs=nci,
                                num_idxs_reg=gather_regs[nci],
                                elem_size=128,
                                queue_num=gather_call_no[0] % 4,
                            )
                            gather_call_no[0] += 1
                            g_chunks[stream].append(gbuf)
                            emitted[stream] = c1

                for w in range(nw):
                    pump_gathers((w + 1) * wb)
                    m_lo = m_pool.tile([128, wcols], bf16, tag="mlo")
                    m_hi = m_pool.tile([128, wcols], bf16, tag="mhi")
                    nc.sync.dma_start(
                        out=m_lo[:], in_=m_d[dirn, "lo"][:, w * wcols:(w + 1) * wcols])
                    nc.sync.dma_start(
                        out=m_hi[:], in_=m_d[dirn, "hi"][:, w * wcols:(w + 1) * wcols])
                    deg_sl = deg_pool.tile([16, wcols], bf16, tag="deg")
                    nc.sync.dma_start(
                        out=deg_sl[:], in_=deg_d[0:16, w * 512:w * 512 + wcols])

                    pw = pagg_pool.tile([128, wcols], f32, tag="pagg")
                    nc.tensor.matmul(
                        pw[:],
                        lhsT=bias_sb[0:16, wl * 128:(wl + 1) * 128],
                        rhs=deg_sl[:],
                        start=True, stop=False, skip_group_check=True,
                    )
                    for stream, m_sl in (("lo", m_lo), ("hi", m_hi)):
                        for t in range(wb):
                            gt = w * wb + t
                            nc.tensor.matmul(
                                pw[:, t * wm:(t + 1) * wm],
                                lhsT=g_chunks[stream][gt // CH][
                                    :, (gt % CH) * 128:(gt % CH + 1) * 128],
                                rhs=m_sl[:, t * wm:(t + 1) * wm],
                                start=False,
                                stop=(stream == "hi" and t == wb - 1),
                                skip_group_check=True,
                            )
                    nc.vector.tensor_copy(dst[w][:, 0:wcols], pw[:])
                    if last:
                        # drain output windows during the remaining agg
                        nc.sync.dma_start(
                            out=out_d[:, w * 512:(w + 1) * 512], in_=dst[w][:])
                    elif w == nw // 2 - 1:
                        # next layer's dense first half only needs windows
                        # 0..nw/2-1; its half-AG then overlaps the remaining
                        # gather windows (it writes the OTHER ping-pong table)
                        emit_dense_stage(dst, layers[li + 1][3], 0, nw)
                        emit_ag(li + 1, 0)
                if not last:
                    emit_dense_stage(dst, layers[li + 1][3], 1, nw)

    nc.compile()
    return nc


# ----------------------------------------------------------------------------
# public entry point
# ----------------------------------------------------------------------------


def kernel(**inputs) -> np.ndarray:
    from concourse.bass_utils import run_bass_kernel_spmd

    n = int(np.asarray(inputs["X_u"]).shape[0])
    loc = n // NC
    # wv: virtual rows per core slice; 4*wv must be >= any lo/hi idx range
    wv = 8192 if n == 50000 else max(512, 1 << (loc * 2 - 1).bit_length())

    packU, packV, per_core = prepare_host_data(inputs, n, loc, wv)
    nc = build_program(packU, packV)
    res = run_bass_kernel_spmd(nc, per_core, list(range(NC)))

    out = np.zeros((n, 128), np.float32)
    for c in range(NC):
        g = np.arange(c * loc, (c + 1) * loc)
        out[g] = res.results[c]["outp"][:, packU.vpos[g]].T.astype(np.float32)
    return out


if __name__ == "__main__":
    data = dict(np.load("/root/problem/inputs_cache.npz"))
    got = kernel(**data)
    np.save("/root/problem/kernel_out.npy", got)
    print("kernel done", got.shape)

